# revision 29
# baseline (speedup 1.0000x reference)
"""Distributed Bass kernel for llama-style GQA attention on 8 trn2 NeuronCores.

Sharding: 2-way data-parallel over batch x 4-way tensor-parallel over heads.
Core c handles batch b=c//4 and head group t=c%4 (8 q-heads, 2 kv-heads).
wq/wk/wv split column-wise per head group; wo split row-wise; each core
produces a partial [S, HIDDEN] output, host sums the 4 partials per batch.

On-chip flow per core (all matmuls bf16, psum f32):
  xT (pre-transposed on host) @ wqkv -> q,k,v  [seq partition-major]
  RoPE on q (pre-scaled by 1/sqrt(D)) and k via even/odd strided APs
  PE-transpose q,k to [d, seq]; v kept [seq, d] with an appended ones column
  scores^T[k,q] = kT.T @ qT ; exp (no max subtraction -- scores are O(5));
  causal via aligned 128x128 tri mask / memset / narrowed score matmuls;
  ctx^T[d,q] accumulated with the ones column giving softmax denominators
  for free; normalize via bf16 K=1 broadcast matmul + fast reciprocal;
  out_partial = ctx^T.T @ wo_shard, split in two halves so the first half
  interleaves with attention (keeps PE dense) via a DRAM scratch.
"""

import numpy as np
import ml_dtypes

import concourse.bass as bass
import concourse.mybir as mybir
import concourse.tile as tile
from concourse import bacc
from concourse.bass_utils import run_bass_kernel_spmd
from concourse.masks import make_identity

B, S, HID = 2, 2048, 2048
D = 64
NQ, NKV = 8, 2          # per-core heads
QW, KW, VW = NQ * D, NKV * D, NKV * D
QKVW = QW + KW + VW     # 768
P = 128
SB = S // P             # 16 seq blocks
KC = HID // P           # 16 contraction chunks
F32 = mybir.dt.float32
BF16 = mybir.dt.bfloat16
BF = ml_dtypes.bfloat16
AF = mybir.ActivationFunctionType

_CACHE = {}


def _emit_graph(nc, tc, xT, wqkv, wo, cos8, sin8, cos1, sin1, out):
    with tc.tile_pool(name="const", bufs=1) as const, \
         tc.tile_pool(name="big", bufs=1) as big, \
         tc.tile_pool(name="dscr", bufs=1, space="DRAM") as dscr:
        # persistent across phases
        # q head pairs: tensor t holds head 2t dims on partitions 0:64, head 2t+1 on 64:128
        qT_sb = [big.tile([P, S], BF16, tag=f"qT{t}", name=f"qT{t}") for t in range(4)]
        # kv head k duplicated on both partition halves (so base partition matches either q half)
        kT_sb = [big.tile([P, S], BF16, tag=f"kT{k}", name=f"kT{k}") for k in range(NKV)]
        vaug_sb = big.tile([P, NKV * SB * 65], BF16, tag="va")
        ctxT_sb = [big.tile([P, S], BF16, tag=f"cT{t}", name=f"cT{t}") for t in range(4)]
        acc_dram = dscr.tile([S, HID], F32, tag="acc")

        ident = const.tile([P, P], BF16, tag="id")
        make_identity(nc, ident[:, :])
        # tri01[k, q] = 1 where q >= k else 0 (keep-mask for aligned diag blocks)
        tri01 = const.tile([P, P], BF16, tag="tri")
        nc.gpsimd.memset(tri01[:, :], 1.0)
        nc.gpsimd.affine_select(
            out=tri01[:, :], in_=tri01[:, :], compare_op=mybir.AluOpType.is_ge,
            fill=0.0, base=0, pattern=[[1, P]], channel_multiplier=-1,
        )
        ones64 = const.tile([1, D], BF16, tag="ones")
        nc.gpsimd.memset(ones64[:, :], 1.0)
        nc.gpsimd.memset(vaug_sb[:, :], 1.0)

        def rope(ps, nh, cos_t, sin_t, dst, sb, tmp_pool):
            """ps: psum [P, nh*64] pre-rotation; dst: sbuf bf16 slice [P, nh*64]."""
            half = nh * 32
            t1 = tmp_pool.tile([P, half], F32, tag="t1", name="t1")
            t2 = tmp_pool.tile([P, half], F32, tag="t2", name="t2")
            ev = ps[:, 0::2].rearrange("p (h i) -> p h i", h=nh)
            od = ps[:, 1::2].rearrange("p (h i) -> p h i", h=nh)
            c = cos_t[:, sb * 32:(sb + 1) * 32].rearrange("p (o i) -> p o i", o=1).broadcast_to((P, nh, 32))
            s = sin_t[:, sb * 32:(sb + 1) * 32].rearrange("p (o i) -> p o i", o=1).broadcast_to((P, nh, 32))
            t1r = t1[:, :].rearrange("p (h i) -> p h i", h=nh)
            t2r = t2[:, :].rearrange("p (h i) -> p h i", h=nh)
            dst_e = dst[:, 0::2].rearrange("p (h i) -> p h i", h=nh)
            dst_o = dst[:, 1::2].rearrange("p (h i) -> p h i", h=nh)
            nc.vector.tensor_mul(t1r, ev, c)
            nc.vector.tensor_mul(t2r, od, s)
            nc.vector.tensor_sub(dst_e, t1r, t2r)
            nc.vector.tensor_mul(t1r, ev, s)
            nc.vector.tensor_mul(t2r, od, c)
            nc.vector.tensor_add(dst_o, t1r, t2r)

        with tc.tile_pool(name="pss", bufs=3, space="PSUM") as pss_p, \
             tc.tile_pool(name="psc", bufs=2, space="PSUM") as psc_p, \
             tc.tile_pool(name="psb", bufs=1, space="PSUM") as psb_p, \
             tc.tile_pool(name="exs", bufs=8) as exs_p, \
             tc.tile_pool(name="nrm", bufs=4) as nrm_p:

            def attn_unit(h, qb):
                t, roff, kv = h // 2, D * (h % 2), h // 4
                qT = qT_sb[t][roff:roff + D, :]
                kT = kT_sb[kv][roff:roff + D, :]
                ctx = psc_p.tile([65, 512], F32, tag="ctx", name="ctx")
                nkb = min(4 * qb + 4, 16)
                for kb in range(nkb):
                    sT = pss_p.tile([P, 512], F32, tag="sT", name="sT")
                    j0 = max(kb - 4 * qb, 0)   # sub-blocks j < j0 are fully masked
                    nc.tensor.matmul(sT[:, j0 * P:512], kT[:, kb * P:(kb + 1) * P],
                                     qT[:, qb * 512 + j0 * P:(qb + 1) * 512], start=True, stop=True)
                    ex = exs_p.tile([P, 512], BF16, tag="ex", name="ex")
                    if j0 > 0:
                        nc.gpsimd.memset(ex[:, 0:j0 * P], 0.0)
                    nc.scalar.activation(ex[:, j0 * P:512], sT[:, j0 * P:512], AF.Exp)
                    if kb * P >= qb * 512 and j0 < 4:   # aligned diagonal sub-block
                        nc.vector.tensor_mul(ex[:, j0 * P:(j0 + 1) * P],
                                             ex[:, j0 * P:(j0 + 1) * P], tri01[:, :])
                    nc.tensor.matmul(ctx[:], vaug_sb[:, kv * SB * 65 + kb * 65: kv * SB * 65 + (kb + 1) * 65],
                                     ex[:], start=(kb == 0), stop=(kb == nkb - 1))
                # ctx rows 0:64 = unnormalized ctx dims; row 64 = softmax denominators
                den = nrm_p.tile([1, 512], BF16, tag="den", name="den")
                nc.vector.tensor_copy(den[:], ctx[64:65, :])
                bc = psb_p.tile([D, 512], F32, tag="bc", name="bc")
                nc.tensor.matmul(bc[:], ones64[:, :], den[:], start=True, stop=True)
                bcs = nrm_p.tile([D, 512], F32, tag="bcs", name="bcs")
                nc.vector.reciprocal_approx_fast(out=bcs[:], in_=bc[:])
                ntmp = nrm_p.tile([D, 512], BF16, tag="ntmp", name="ntmp")
                nc.vector.tensor_mul(ntmp[:], ctx[0:D, :], bcs[:])
                nc.sync.dma_start(out=ctxT_sb[t][roff:roff + D, qb * 512:(qb + 1) * 512],
                                  in_=ntmp[:])

            # ---- projections: kv first, then q in 2 subgroups of 4 heads ----
            # wqkv columns: [k0|k1 (128) | v0|v1 (128) | q0..q3 (256) | q4..q7 (256)]
            with tc.tile_pool(name="p1", bufs=1) as p1, \
                 tc.tile_pool(name="psg", bufs=2, space="PSUM") as psg_p, \
                 tc.tile_pool(name="rtmp", bufs=2) as rtmp_p:
                pst_p = psb_p
                xT_sb = p1.tile([P, KC * S], BF16, tag="xT")
                wqkv_sb = p1.tile([P, KC * QKVW], BF16, tag="wqkv")
                cos8_sb = p1.tile([P, SB * 32], F32, tag="c8")
                sin8_sb = p1.tile([P, SB * 32], F32, tag="s8")
                cos1_sb = p1.tile([P, SB * 32], F32, tag="c1")
                sin1_sb = p1.tile([P, SB * 32], F32, tag="s1")
                qrot_sb = p1.tile([P, SB * 256], BF16, tag="qr")
                krot_sb = p1.tile([P, SB * 128], BF16, tag="kr")
                for kc in range(KC):
                    nc.sync.dma_start(out=xT_sb[:, kc * S:(kc + 1) * S], in_=xT[kc * P:(kc + 1) * P, :])
                    nc.sync.dma_start(out=wqkv_sb[:, kc * QKVW:(kc + 1) * QKVW], in_=wqkv[kc * P:(kc + 1) * P, :])
                for sb in range(SB):
                    for dst, srcz in ((cos8_sb, cos8), (sin8_sb, sin8), (cos1_sb, cos1), (sin1_sb, sin1)):
                        nc.sync.dma_start(out=dst[:, sb * 32:(sb + 1) * 32], in_=srcz[sb * P:(sb + 1) * P, :])

                def kv_block(sb):
                    ps = psg_p.tile([P, 256], F32, tag="psg", name="psg")
                    for kc in range(KC):
                        nc.tensor.matmul(ps[:], xT_sb[:, kc * S + sb * P: kc * S + (sb + 1) * P],
                                         wqkv_sb[:, kc * QKVW: kc * QKVW + 256],
                                         start=(kc == 0), stop=(kc == KC - 1))
                    rope(ps[:, 0:KW], NKV, cos1_sb, sin1_sb, krot_sb[:, sb * KW:(sb + 1) * KW], sb, rtmp_p)
                    for kv in range(NKV):
                        nc.vector.tensor_copy(
                            vaug_sb[:, kv * SB * 65 + sb * 65: kv * SB * 65 + sb * 65 + 64],
                            ps[:, KW + kv * D: KW + (kv + 1) * D])
                    pt = pst_p.tile([P, P], BF16, tag="bc", name="pt")
                    nc.tensor.transpose(pt[:], krot_sb[:, sb * KW:(sb + 1) * KW], ident[:, :])
                    # kv0 dims land on partitions 0:64, kv1 on 64:128; write each half
                    nc.vector.tensor_copy(kT_sb[0][0:D, sb * P:(sb + 1) * P], pt[0:D, :])
                    nc.vector.tensor_copy(kT_sb[1][D:P, sb * P:(sb + 1) * P], pt[D:P, :])
                    nc.sync.dma_start(out=kT_sb[0][D:P, sb * P:(sb + 1) * P],
                                      in_=kT_sb[0][0:D, sb * P:(sb + 1) * P])
                    nc.sync.dma_start(out=kT_sb[1][0:D, sb * P:(sb + 1) * P],
                                      in_=kT_sb[1][D:P, sb * P:(sb + 1) * P])

                def q_block(j, sb):
                    ps = psg_p.tile([P, 256], F32, tag="psg", name="psg")
                    for kc in range(KC):
                        nc.tensor.matmul(ps[:], xT_sb[:, kc * S + sb * P: kc * S + (sb + 1) * P],
                                         wqkv_sb[:, kc * QKVW + 256 + j * 256: kc * QKVW + 256 + (j + 1) * 256],
                                         start=(kc == 0), stop=(kc == KC - 1))
                    rope(ps[:], 4, cos8_sb, sin8_sb, qrot_sb[:, sb * 256:(sb + 1) * 256], sb, rtmp_p)
                    for pidx in range(2):
                        t = 2 * j + pidx
                        pt = pst_p.tile([P, P], BF16, tag="bc", name="pt")
                        nc.tensor.transpose(pt[:], qrot_sb[:, sb * 256 + pidx * P: sb * 256 + (pidx + 1) * P],
                                            ident[:, :])
                        nc.vector.tensor_copy(qT_sb[t][:, sb * P:(sb + 1) * P], pt[:])

                for sb in range(SB):
                    kv_block(sb)
                for sb in range(SB):
                    q_block(0, sb)
                # q subgroup 1 interleaved with attention over subgroup-0 heads
                g0_units = [(h, qb) for h in range(4) for qb in range(4)]
                for sb in range(SB):
                    q_block(1, sb)
                    attn_unit(*g0_units[sb])

            # ---- attention subgroup 1 interleaved with first half of wo ----
            with tc.tile_pool(name="pso", bufs=2, space="PSUM") as pso_p, \
                 tc.tile_pool(name="osb", bufs=3) as osb_p, \
                 tc.tile_pool(name="wop", bufs=1) as wop:
                wo_sb = wop.tile([P, 4 * HID], BF16, tag="wo")
                for c in range(4):
                    nc.sync.dma_start(out=wo_sb[:, c * HID:(c + 1) * HID], in_=wo[c * P:(c + 1) * P, :])

                def wo_half(sb, n, cs, dst_ap, accum_sb=None):
                    po = pso_p.tile([P, 512], F32, tag="po", name="po")
                    for i, c in enumerate(cs):
                        nc.tensor.matmul(po[:], ctxT_sb[c][:, sb * P:(sb + 1) * P],
                                         wo_sb[:, c * HID + n * 512: c * HID + (n + 1) * 512],
                                         start=(i == 0), stop=(i == len(cs) - 1))
                    ob = osb_p.tile([P, 512], F32, tag="ob", name="ob")
                    if accum_sb is None:
                        nc.vector.tensor_copy(ob[:], po[:])
                    else:
                        nc.vector.tensor_add(ob[:], po[:], accum_sb[:])
                    nc.sync.dma_start(out=dst_ap, in_=ob[:])

                def wo_B(sb, n):
                    acc_sb = osb_p.tile([P, 512], F32, tag="acl", name="acl")
                    nc.sync.dma_start(out=acc_sb[:], in_=acc_dram[sb * P:(sb + 1) * P, n * 512:(n + 1) * 512])
                    wo_half(sb, n, (2, 3), out[sb * P:(sb + 1) * P, n * 512:(n + 1) * 512], accum_sb=acc_sb)

                # qb-major: after each qb's 4 heads, ctxT columns for 4 seq
                # blocks are final -> their wo halves interleave right here
                for qb in range(4):
                    for h in range(4, 8):
                        attn_unit(h, qb)
                    for sb in range(4 * qb, 4 * qb + 4):
                        for n in range(4):
                            wo_half(sb, n, (0, 1), acc_dram[sb * P:(sb + 1) * P, n * 512:(n + 1) * 512])
                    if qb > 0:
                        for sb in range(4 * (qb - 1), 4 * qb):
                            for n in range(4):
                                wo_B(sb, n)
                for sb in range(12, 16):
                    for n in range(4):
                        wo_B(sb, n)


def _build():
    nc = bacc.Bacc("TRN2", target_bir_lowering=False, debug=False, num_devices=8)
    xT = nc.dram_tensor("xT", [HID, S], BF16, kind="ExternalInput").ap()
    wqkv = nc.dram_tensor("wqkv", [HID, QKVW], BF16, kind="ExternalInput").ap()
    wo = nc.dram_tensor("wo", [QW, HID], BF16, kind="ExternalInput").ap()
    cos8 = nc.dram_tensor("cos8", [S, 32], F32, kind="ExternalInput").ap()
    sin8 = nc.dram_tensor("sin8", [S, 32], F32, kind="ExternalInput").ap()
    cos1 = nc.dram_tensor("cos1", [S, 32], F32, kind="ExternalInput").ap()
    sin1 = nc.dram_tensor("sin1", [S, 32], F32, kind="ExternalInput").ap()
    out = nc.dram_tensor("out", [S, HID], F32, kind="ExternalOutput").ap()
    with tile.TileContext(nc) as tc:
        _emit_graph(nc, tc, xT, wqkv, wo, cos8, sin8, cos1, sin1, out)
    nc.finalize()
    return nc


def kernel(x, wq, wk, wv, wo, freqs_cos, freqs_sin, mask):
    x = np.asarray(x, dtype=np.float32)
    wq = np.asarray(wq, dtype=np.float32)
    wk = np.asarray(wk, dtype=np.float32)
    wv = np.asarray(wv, dtype=np.float32)
    wo = np.asarray(wo, dtype=np.float32)
    fc = np.asarray(freqs_cos, dtype=np.float32)
    fs = np.asarray(freqs_sin, dtype=np.float32)

    if "nc" not in _CACHE:
        _CACHE["nc"] = _build()
    nc = _CACHE["nc"]

    inv = 1.0 / np.sqrt(np.float32(D))
    cos8 = np.ascontiguousarray(fc * inv)
    sin8 = np.ascontiguousarray(fs * inv)
    in_maps = []
    for core in range(8):
        b, t = core // 4, core % 4
        in_maps.append({
            "xT": np.ascontiguousarray(x[b].T).astype(BF),
            "wqkv": np.ascontiguousarray(np.concatenate(
                [wk[:, t * KW:(t + 1) * KW],
                 wv[:, t * VW:(t + 1) * VW],
                 wq[:, t * QW:(t + 1) * QW]], axis=1)).astype(BF),
            "wo": np.ascontiguousarray(wo[t * QW:(t + 1) * QW, :]).astype(BF),
            "cos8": cos8, "sin8": sin8,
            "cos1": np.ascontiguousarray(fc), "sin1": np.ascontiguousarray(fs),
        })
    trace = bool(_CACHE.get("trace"))
    try:
        res = run_bass_kernel_spmd(nc, in_maps, list(range(8)), trace=trace)
    except Exception:
        if not trace:
            raise
        res = run_bass_kernel_spmd(nc, in_maps, list(range(8)))
    _CACHE["last_result"] = res
    outs = [np.asarray(r["out"], dtype=np.float32) for r in res.results]
    full = np.stack([outs[0] + outs[1] + outs[2] + outs[3],
                     outs[4] + outs[5] + outs[6] + outs[7]], axis=0)
    return full
